# revision 39
# baseline (speedup 1.0000x reference)
"""BatchHardTripletMarginLoss on 8 Trainium2 NeuronCores.

Strategy (anchor-sharded, embeddings replicated):
  - Each of the 8 cores owns 512 anchor rows of the 4096x4096 distance matrix.
  - Per [128,512] tile, PE computes psum = -2*G + (sq_j + S) [+ BIG*same for
    the label fast path] with fp32r matmuls.  Row-constant terms (sq_i) don't
    change per-row argmax/argmin, so they're folded back in on [128,4] tails.
  - Hard mining per row: chunk max (pos needle) / chunk min (neg needle),
    then one DVE max8/find_index8 pass recovers first-occurrence indices
    (== jnp.argmax tie semantics).
  - e[hp], e[hn] gathered via indirect DMA; d_pn^2 = |e_hp - e_hn|^2 exactly.
  - Per-core output: [128, 2] partial (sum of nonzero losses, count); host
    reduces across cores/partitions.

Three device variants:
  label:    masks are label-derived (pos = same & ~eye, neg = ~same, #classes
            <= 126).  The +BIG pos offset rides the matmul via one-hot rows;
            no mask DMA at all.  (This is what setup_inputs() produces.)
  disjoint: arbitrary masks with pos & neg disjoint: one combined bf16
            additive mask (+BIG pos / 0 neg / +MID neither).
  overlap:  fully general fallback (two additive masks, two match passes).
"""

import math
import os

import numpy as np
import ml_dtypes

N, D, NCORES = 4096, 256, 8
R = N // NCORES          # 512 anchors per core
P = 128                  # partitions
NBLK = R // P            # 4 anchor blocks per core
CW = 512                 # chunk width (psum free dim)
NCH = N // CW            # 8 chunks per row
MARGIN = 0.2
MAXC = 126               # max classes for the label path (K=1+C <= 128)
DEBUG = False            # emit dbg/dbgi outputs (sim validation)

_CACHE = {}


def _build(mode: str, mm_dtype_name: str, nclass: int = 0):
    import concourse.bass as bass
    import concourse.mybir as mybir
    import concourse.tile as tile
    from concourse import bacc

    f32 = mybir.dt.float32
    bf16 = mybir.dt.bfloat16
    u32 = mybir.dt.uint32
    mmdt = getattr(mybir.dt, mm_dtype_name)
    Alu = mybir.AluOpType
    Act = mybir.ActivationFunctionType

    label = mode == "label"
    disjoint = mode != "overlap"
    KA = 1 + nclass if label else 1  # contraction rows of the 3rd matmul

    nc = bacc.Bacc("TRN2", target_bir_lowering=False, debug=False, num_devices=NCORES)

    # PE-consumed operands, grouped so early chunks' data lands first:
    #  pesm [P, 3R]: (-2 E_blk)^T rows 0:128 | rows 128:256 | AUGL
    #    (AUGL: row0 ones; rows 1..C = BIG*U_blk^T)
    #  etq  [P, 2N]: E^T rows 0:128 | rows 128:256 (DMA'd in quarters)
    #  augr [P, N]:  row0 sq_j+S; rows 1..C = U^T (DMA'd in quarters)
    pesm_d = nc.dram_tensor("pesm", [P, 3 * R], mmdt, kind="ExternalInput")  # block-packed
    etq_d = nc.dram_tensor("etq", [P, 2 * N], mmdt, kind="ExternalInput")
    augr_d = nc.dram_tensor("augr", [P, N], mmdt, kind="ExternalInput")
    QW = N // 4  # 1024: quarter width
    e_full = nc.dram_tensor("e", [N, D], f32, kind="ExternalInput")
    sqi_ap = nc.dram_tensor("sqi_ap", [P, NBLK], f32, kind="ExternalInput")
    sqi_an = nc.dram_tensor("sqi_an", [P, NBLK], f32, kind="ExternalInput")
    thr = nc.dram_tensor("thr", [P, 2], f32, kind="ExternalInput")
    if label:
        own_idx = nc.dram_tensor("own_idx", [P, NBLK], u32, kind="ExternalInput")
    elif disjoint:
        mc_d = nc.dram_tensor("mc", [R, N], bf16, kind="ExternalInput")
    else:
        mcp_d = nc.dram_tensor("mcp", [R, N], bf16, kind="ExternalInput")
        mcn_d = nc.dram_tensor("mcn", [R, N], bf16, kind="ExternalInput")
    out_d = nc.dram_tensor("out", [P, 2], f32, kind="ExternalOutput")
    if DEBUG:
        dbg_d = nc.dram_tensor("dbg", [P, NBLK * 4], f32, kind="ExternalOutput")
        dbgi_d = nc.dram_tensor("dbgi", [P, NBLK * 2], mybir.dt.uint32, kind="ExternalOutput")

    with tile.TileContext(nc) as tc:
        with (
            tc.tile_pool(name="consts", bufs=1) as consts,
            tc.tile_pool(name="masks", bufs=2) as maskp,
            tc.tile_pool(name="cands", bufs=3) as candp,
            tc.tile_pool(name="psum", bufs=8, space="PSUM") as psump,
            tc.tile_pool(name="small", bufs=6) as smallp,
            tc.tile_pool(name="gath", bufs=4) as gathp,
            tc.tile_pool(name="tail", bufs=1) as tailp,
        ):
            pesm_blks = [consts.tile([P, 3 * P], mmdt, tag=f"pesm{b}", name=f"pesm{b}") for b in range(NBLK)]
            et0q = [consts.tile([P, QW], mmdt, tag=f"et0q{q}", name=f"et0q{q}") for q in range(4)]
            et1q = [consts.tile([P, QW], mmdt, tag=f"et1q{q}", name=f"et1q{q}") for q in range(4)]
            augrq = [consts.tile([P, QW], mmdt, tag=f"augrq{q}", name=f"augrq{q}") for q in range(4)]

            def _dma_q(q):
                nc.sync.dma_start(out=et0q[q][:], in_=etq_d[:, q * QW : (q + 1) * QW])
                nc.sync.dma_start(out=et1q[q][:], in_=etq_d[:, N + q * QW : N + (q + 1) * QW])
                nc.sync.dma_start(out=augrq[q][0:KA, :], in_=augr_d[0:KA, q * QW : (q + 1) * QW])

            nc.sync.dma_start(out=pesm_blks[0][:], in_=pesm_d[:, 0 : 3 * P])
            _dma_q(0)
            for b in range(1, NBLK):
                nc.sync.dma_start(out=pesm_blks[b][:], in_=pesm_d[:, b * 3 * P : (b + 1) * 3 * P])
            for q in range(1, 4):
                _dma_q(q)
            sqi_ap_sb = consts.tile([P, NBLK], f32, tag="sqi_ap")
            nc.sync.dma_start(out=sqi_ap_sb[:], in_=sqi_ap[:])
            sqi_an_sb = consts.tile([P, NBLK], f32, tag="sqi_an")
            nc.sync.dma_start(out=sqi_an_sb[:], in_=sqi_an[:])
            thr_sb = consts.tile([P, 2], f32, tag="thr")
            nc.sync.dma_start(out=thr_sb[:], in_=thr[:])
            if label:
                own_sb = consts.tile([P, NBLK], u32, tag="own")
                nc.sync.dma_start(out=own_sb[:], in_=own_idx[:])

            pmax_arr = tailp.tile([P, NBLK], f32, tag="pmax_arr")
            nmin_arr = tailp.tile([P, NBLK], f32, tag="nmin_arr")
            dpn2 = tailp.tile([P, NBLK], f32, tag="dpn2")
            dbgi_sb = tailp.tile([P, NBLK * 2], u32, tag="dbgi_sb")
            if label:
                hpi = tailp.tile([P, NBLK], u32, tag="hpi")

            for b in range(NBLK):
                rows = slice(b * P, (b + 1) * P)
                if not label:
                    if disjoint:
                        mc_b = maskp.tile([P, N], bf16, tag="mc")
                        nc.sync.dma_start(out=mc_b[:], in_=mc_d[rows, :])
                    else:
                        mcp_b = maskp.tile([P, N], bf16, tag="mcp")
                        nc.sync.dma_start(out=mcp_b[:], in_=mcp_d[rows, :])
                        mcn_b = maskp.tile([P, N], bf16, tag="mcn")
                        nc.sync.dma_start(out=mcn_b[:], in_=mcn_d[rows, :])

                cand_b = candp.tile([P, N], f32, tag="cand")
                if not disjoint:
                    ncand_b = candp.tile([P, N], f32, tag="ncand")
                pm = smallp.tile([P, NCH], f32, tag="pm")
                nm = smallp.tile([P, NCH], f32, tag="nm")

                lhs0 = pesm_blks[b][:, 0:P]
                lhs1 = pesm_blks[b][:, P : 2 * P]
                lhsa = pesm_blks[b][0:KA, 2 * P : 3 * P]
                for c in range(NCH):
                    cs = slice(c * CW, (c + 1) * CW)
                    q, qs = c // 2, slice((c % 2) * CW, (c % 2) * CW + CW)
                    ps = psump.tile([P, CW], f32, tag="ps")
                    nc.tensor.matmul(out=ps[:], lhsT=lhs0, rhs=et0q[q][:, qs], start=True, stop=False)
                    nc.tensor.matmul(out=ps[:], lhsT=lhs1, rhs=et1q[q][:, qs], start=False, stop=False)
                    nc.tensor.matmul(out=ps[:], lhsT=lhsa, rhs=augrq[q][0:KA, qs], start=False, stop=True)
                    if label:
                        # psum IS the candidate array (mask offsets ride the
                        # matmul).  ACT evicts for the match; DVE mines the
                        # SBUF copy in chunk pairs (one op per 2 chunks).
                        nc.scalar.activation(cand_b[:, cs], ps[:], Act.Copy)
                        if b == 0 and c <= 1:
                            # first block-pair: mine each chunk immediately so
                            # DVE starts as soon as the first evict lands
                            one = cand_b[:, cs].rearrange("p (t w) -> p t w", t=1)
                            nc.vector.tensor_reduce(
                                out=pm[:, c : c + 1], in_=one,
                                axis=mybir.AxisListType.X, op=Alu.max,
                            )
                            nc.vector.tensor_reduce(
                                out=nm[:, c : c + 1], in_=one,
                                axis=mybir.AxisListType.X, op=Alu.min,
                            )
                        elif c % 2 == 1:
                            pair = cand_b[:, (c - 1) * CW : (c + 1) * CW].rearrange(
                                "p (t w) -> p t w", t=2
                            )
                            nc.vector.tensor_reduce(
                                out=pm[:, c - 1 : c + 1], in_=pair,
                                axis=mybir.AxisListType.X, op=Alu.max,
                            )
                            nc.vector.tensor_reduce(
                                out=nm[:, c - 1 : c + 1], in_=pair,
                                axis=mybir.AxisListType.X, op=Alu.min,
                            )
                    elif disjoint:
                        nc.vector.tensor_tensor(
                            out=cand_b[:, cs], in0=ps[:], in1=mc_b[:, cs], op=Alu.add
                        )
                        nc.vector.tensor_reduce(
                            out=pm[:, c : c + 1], in_=cand_b[:, cs],
                            axis=mybir.AxisListType.X, op=Alu.max,
                        )
                        nc.vector.tensor_reduce(
                            out=nm[:, c : c + 1], in_=cand_b[:, cs],
                            axis=mybir.AxisListType.X, op=Alu.min,
                        )
                    else:
                        nc.vector.tensor_tensor(
                            out=cand_b[:, cs], in0=ps[:], in1=mcp_b[:, cs], op=Alu.add
                        )
                        nc.vector.tensor_reduce(
                            out=pm[:, c : c + 1], in_=cand_b[:, cs],
                            axis=mybir.AxisListType.X, op=Alu.max,
                        )
                        nc.vector.tensor_tensor(
                            out=ncand_b[:, cs], in0=ps[:], in1=mcn_b[:, cs], op=Alu.add
                        )
                        nc.vector.tensor_reduce(
                            out=nm[:, c : c + 1], in_=ncand_b[:, cs],
                            axis=mybir.AxisListType.X, op=Alu.min,
                        )

                psort = smallp.tile([P, 8], f32, tag="psort")
                nsort = smallp.tile([P, 8], f32, tag="nsort")
                nc.vector.max(psort[:], pm[:])
                nc.vector.max(nsort[:], nm[:])
                nc.scalar.activation(pmax_arr[:, b : b + 1], psort[:, 0:1], Act.Copy)
                nc.scalar.activation(nmin_arr[:, b : b + 1], nsort[:, 7:8], Act.Copy)

                if disjoint:
                    comb = smallp.tile([P, 8], f32, tag="comb")
                    nc.vector.tensor_copy(comb[:], psort[:])
                    nc.vector.tensor_copy(comb[:, 1:2], nsort[:, 7:8])
                    idx8 = smallp.tile([P, 8], u32, tag="idx8")
                    nc.vector.max_index(idx8[:], comb[:], cand_b[:])
                    hp_idx = idx8[:, 0:1]
                    hn_idx = idx8[:, 1:2]
                else:
                    pidx8 = smallp.tile([P, 8], u32, tag="pidx8")
                    nc.vector.max_index(pidx8[:], psort[:], cand_b[:])
                    nidx8 = smallp.tile([P, 8], u32, tag="nidx8")
                    nc.vector.max_index(nidx8[:], nsort[:], ncand_b[:])
                    hp_idx = pidx8[:, 0:1]
                    hn_idx = nidx8[:, 7:8]

                # clamp indices (unmatched needles return -1 == u32 max)
                idxc = smallp.tile([P, 2], u32, tag="idxc")
                nc.vector.tensor_scalar_min(idxc[:, 0:1], hp_idx, N - 1)
                nc.vector.tensor_scalar_min(idxc[:, 1:2], hn_idx, N - 1)
                if label:
                    nc.vector.tensor_copy(hpi[:, b : b + 1], idxc[:, 0:1])
                if DEBUG:
                    nc.vector.tensor_copy(dbgi_sb[:, b : b + 1], idxc[:, 0:1])
                    nc.vector.tensor_copy(dbgi_sb[:, NBLK + b : NBLK + b + 1], idxc[:, 1:2])
                # gather e[hp], e[hn]; d_pn^2 = |e_hp - e_hn|^2
                ehp = gathp.tile([P, D], f32, tag="ehp")
                ehn = gathp.tile([P, D], f32, tag="ehn")
                nc.gpsimd.indirect_dma_start(
                    out=ehp[:], out_offset=None, in_=e_full[:],
                    in_offset=bass.IndirectOffsetOnAxis(ap=idxc[:, 0:1], axis=0),
                )
                nc.gpsimd.indirect_dma_start(
                    out=ehn[:], out_offset=None, in_=e_full[:],
                    in_offset=bass.IndirectOffsetOnAxis(ap=idxc[:, 1:2], axis=0),
                )
                diff = gathp.tile([P, D], f32, tag="diff")
                scr = gathp.tile([P, D], f32, tag="scr")
                nc.gpsimd.tensor_tensor(
                    out=diff[:], in0=ehp[:], in1=ehn[:], op=Alu.subtract
                )
                nc.gpsimd.tensor_tensor(
                    out=scr[:], in0=diff[:], in1=diff[:], op=Alu.mult
                )
                nc.vector.tensor_reduce(
                    out=dpn2[:, b : b + 1], in_=scr[:],
                    axis=mybir.AxisListType.X, op=Alu.add,
                )

            # ---- tail: per-anchor losses ([P, NBLK] arrays) ----
            dap2 = tailp.tile([P, NBLK], f32, tag="dap2")
            dan2 = tailp.tile([P, NBLK], f32, tag="dan2")
            nc.vector.tensor_tensor(out=dap2[:], in0=pmax_arr[:], in1=sqi_ap_sb[:], op=Alu.add)
            nc.vector.tensor_tensor(out=dan2[:], in0=nmin_arr[:], in1=sqi_an_sb[:], op=Alu.add)
            nc.vector.tensor_scalar_max(dap2[:], dap2[:], 0.0)
            nc.vector.tensor_scalar_max(dan2[:], dan2[:], 0.0)
            dane2 = tailp.tile([P, NBLK], f32, tag="dane2")
            nc.vector.tensor_tensor(out=dane2[:], in0=dan2[:], in1=dpn2[:], op=Alu.min)
            dap = tailp.tile([P, NBLK], f32, tag="dap")
            dane = tailp.tile([P, NBLK], f32, tag="dane")
            nc.scalar.activation(dap[:], dap2[:], Act.Sqrt)
            nc.scalar.activation(dane[:], dane2[:], Act.Sqrt)
            s_t = tailp.tile([P, NBLK], f32, tag="s_t")
            nc.vector.tensor_tensor(out=s_t[:], in0=dap[:], in1=dane[:], op=Alu.subtract)
            nc.vector.tensor_scalar_add(s_t[:], s_t[:], MARGIN)
            l_t = tailp.tile([P, NBLK], f32, tag="l_t")
            nc.vector.tensor_scalar_max(l_t[:], s_t[:], 0.0)
            gt = tailp.tile([P, NBLK], f32, tag="gt")
            nc.vector.tensor_scalar(
                out=gt[:], in0=s_t[:], scalar1=0.0, scalar2=None, op0=Alu.is_gt
            )
            vp = tailp.tile([P, NBLK], f32, tag="vp")
            vn = tailp.tile([P, NBLK], f32, tag="vn")
            if label:
                # valid_pos <=> hp != own column (self wins iff no positive)
                vpu = tailp.tile([P, NBLK], u32, tag="vpu")
                nc.vector.tensor_tensor(out=vpu[:], in0=hpi[:], in1=own_sb[:], op=Alu.not_equal)
                nc.vector.tensor_copy(vp[:], vpu[:])
            else:
                nc.vector.tensor_scalar(
                    out=vp[:], in0=pmax_arr[:], scalar1=thr_sb[:, 0:1], scalar2=None, op0=Alu.is_ge
                )
            nc.vector.tensor_scalar(
                out=vn[:], in0=nmin_arr[:], scalar1=thr_sb[:, 1:2], scalar2=None, op0=Alu.is_le
            )
            valid = tailp.tile([P, NBLK], f32, tag="valid")
            nc.vector.tensor_tensor(out=valid[:], in0=vp[:], in1=vn[:], op=Alu.mult)
            contrib = tailp.tile([P, NBLK], f32, tag="contrib")
            nc.vector.tensor_tensor(out=contrib[:], in0=l_t[:], in1=valid[:], op=Alu.mult)
            cntc = tailp.tile([P, NBLK], f32, tag="cntc")
            nc.vector.tensor_tensor(out=cntc[:], in0=gt[:], in1=valid[:], op=Alu.mult)

            if DEBUG:
                dbg_sb = tailp.tile([P, NBLK * 4], f32, tag="dbg_sb")
                nc.scalar.activation(dbg_sb[:, 0:NBLK], pmax_arr[:], Act.Copy)
                nc.scalar.activation(dbg_sb[:, NBLK : 2 * NBLK], nmin_arr[:], Act.Copy)
                nc.scalar.activation(dbg_sb[:, 2 * NBLK : 3 * NBLK], dpn2[:], Act.Copy)
                nc.scalar.activation(dbg_sb[:, 3 * NBLK : 4 * NBLK], dap2[:], Act.Copy)
                nc.sync.dma_start(out=dbg_d[:], in_=dbg_sb[:])
                nc.sync.dma_start(out=dbgi_d[:], in_=dbgi_sb[:])

            out_sb = tailp.tile([P, 2], f32, tag="out_sb")
            nc.vector.tensor_reduce(
                out=out_sb[:, 0:1], in_=contrib[:], axis=mybir.AxisListType.X, op=Alu.add
            )
            nc.vector.tensor_reduce(
                out=out_sb[:, 1:2], in_=cntc[:], axis=mybir.AxisListType.X, op=Alu.add
            )
            nc.sync.dma_start(out=out_d[:], in_=out_sb[:])

    nc.finalize()
    return nc


def _next_pow2(x: float) -> float:
    return float(2.0 ** math.ceil(math.log2(max(x, 1.0))))


def _detect_labels(pos: np.ndarray, neg: np.ndarray):
    """If (pos, neg) are label-derived (pos = same&~eye, neg = ~same) with
    <= MAXC classes, return int labels [N]; else None."""
    if pos.diagonal().any():
        return None
    same = pos.copy()
    np.fill_diagonal(same, True)
    if np.logical_xor(neg, ~same).any():
        return None
    lab = np.argmax(same, axis=1)  # first member of each row's class
    if not np.array_equal(same, lab[:, None] == lab[None, :]):
        return None
    if len(np.unique(lab)) > MAXC:
        return None
    return lab


def prep(embeddings, positives_mask, negatives_mask):
    """Host-side prep shared by kernel(), test.py and profile.py.
    Returns (mode, nclass, in_maps)."""
    emb = np.ascontiguousarray(embeddings, dtype=np.float32)
    pos = np.asarray(positives_mask).astype(bool)
    neg = np.asarray(negatives_mask).astype(bool)

    sq = (emb.astype(np.float64) ** 2).sum(axis=1).astype(np.float32)
    smax = float(sq.max())
    S = _next_pow2(smax)
    r_hi = 4.0 * smax + S
    MID = _next_pow2(r_hi * 1.1 + 4.0)

    lab = _detect_labels(pos, neg)
    if lab is not None:
        mode = "label"
        BIG = _next_pow2(2.0 * r_hi + 64.0)
        t_pos = 0.0  # unused (validity via hp != own)
        t_neg = (r_hi + BIG) / 2.0
        uniq = np.unique(lab)
        nclass = len(uniq)
        U = (lab[:, None] == uniq[None, :]).astype(np.float32)  # [N, C]
    else:
        nclass = 0
        BIG = 4.0 * MID
        if not bool(np.logical_and(pos, neg).any()):
            mode = "disjoint"
            t_pos, t_neg = 2.0 * MID, (MID + r_hi) / 2.0
        else:
            mode = "overlap"
            t_pos, t_neg = -BIG / 2.0, BIG / 2.0

    et = np.ascontiguousarray(emb.T)
    sqjs = (sq + np.float32(S)).astype(np.float32)
    bf = ml_dtypes.bfloat16
    if mode == "disjoint":
        mc_full = np.where(
            pos, np.float32(BIG), np.where(neg, np.float32(0.0), np.float32(MID))
        ).astype(bf)
    elif mode == "overlap":
        mcp_full = np.where(pos, np.float32(0.0), np.float32(-BIG)).astype(bf)
        mcn_full = np.where(neg, np.float32(0.0), np.float32(BIG)).astype(bf)

    thr = np.empty((P, 2), np.float32)
    thr[:, 0] = t_pos
    thr[:, 1] = t_neg

    etq = np.concatenate([et[0:P], et[P : 2 * P]], axis=1)  # [P, 2N], shared
    augr = np.zeros((P, N), np.float32)
    augr[0] = sqjs
    if mode == "label":
        augr[1 : 1 + nclass] = U.T
    in_maps = []
    for c in range(NCORES):
        rows = slice(c * R, (c + 1) * R)
        sqi = sq[rows].reshape(NBLK, P).T.copy()  # [P, NBLK]
        pesm = np.zeros((P, 3 * R), np.float32)
        etm2 = (-2.0 * emb[rows]).T.astype(np.float32)
        for b in range(NBLK):
            bs = slice(b * P, (b + 1) * P)
            o = b * 3 * P
            pesm[:, o : o + P] = etm2[0:P, bs]
            pesm[:, o + P : o + 2 * P] = etm2[P : 2 * P, bs]
            pesm[0, o + 2 * P : o + 3 * P] = 1.0
            if mode == "label":
                pesm[1 : 1 + nclass, o + 2 * P : o + 3 * P] = (
                    np.float32(BIG) * U[c * R + b * P : c * R + (b + 1) * P].T
                )
        m = {
            "pesm": pesm,
            "etq": etq,
            "augr": augr,
            "e": emb,
            "sqi_an": sqi - np.float32(S),
            "thr": thr,
        }
        if mode == "label":
            m["sqi_ap"] = sqi - np.float32(S + BIG)
            m["own_idx"] = (
                np.arange(c * R, (c + 1) * R, dtype=np.uint32).reshape(NBLK, P).T.copy()
            )
        elif mode == "disjoint":
            m["sqi_ap"] = sqi - np.float32(S + BIG)
            m["mc"] = np.ascontiguousarray(mc_full[rows])
        else:
            m["sqi_ap"] = sqi - np.float32(S)
            m["mcp"] = np.ascontiguousarray(mcp_full[rows])
            m["mcn"] = np.ascontiguousarray(mcn_full[rows])
        in_maps.append(m)
    return mode, nclass, in_maps


def kernel(embeddings: np.ndarray, positives_mask: np.ndarray, negatives_mask: np.ndarray) -> np.ndarray:
    from concourse.bass_utils import run_bass_kernel_spmd

    mode, nclass, in_maps = prep(embeddings, positives_mask, negatives_mask)
    mm_dtype = os.environ.get("BHK_MM_DTYPE", "float32r")
    key = (mode, mm_dtype, nclass, DEBUG)
    if key not in _CACHE:
        _CACHE[key] = _build(mode, mm_dtype, nclass)
    nc = _CACHE[key]

    res = run_bass_kernel_spmd(nc, in_maps, core_ids=list(range(NCORES)))
    total = 0.0
    cnt = 0.0
    for r in res.results:
        o = r["out"]
        total += float(o[:, 0].sum(dtype=np.float64))
        cnt += float(o[:, 1].sum(dtype=np.float64))
    val = np.float32(total / cnt) if cnt > 0 else np.float32(0.0)
    return np.array(val, dtype=np.float32)
